# revision 1
# baseline (speedup 1.0000x reference)
"""LowRankGlobalAttention TRN2 Bass kernel (8-core SPMD).

out = concat(relu(xW+b)[:, :32] @ (V^T Z) * D, T) where
U,V,Z,T = relu(xW+b) column blocks, D = 1/(sum(U @ colsum(V))/N + eps).

Strategy per core (row-sharded, 62500 rows each):
  Phase 1: stream x in 512-row supertiles; PE-transpose x to put d on
  partitions; W-stationary fp32r GEMM produces X.T [cols, rows]; relu on
  ACT (bias per-partition, accum_out gives colsums); V/Z chunks
  transposed back for the VtZ PSUM-accumulated matmul; U.T/T.T halves
  parked in SBUF.
  AllReduce tiny stats; compute D on-chip.
  Phase 2: res.T = VtZ_D.T-weighted matmul over stored U.T; PE-transpose
  [res.T;T.T] back to row-major and DMA out.
"""
import os
import numpy as np

import concourse.bass as bass
import concourse.mybir as mybir
import concourse.tile as tile
from concourse import bacc
from concourse.bass_utils import run_bass_kernel_spmd
from concourse.masks import make_identity

F32 = mybir.dt.float32
F32R = mybir.dt.float32r

N_CORES = 8
N_TOTAL = 500000
NR = N_TOTAL // N_CORES          # 62500 rows per core
D_IN = 256
KATT = 32
R = 512                          # supertile rows
G = 4                            # row-blocks per supertile (R/128)
NS = NR // R                     # 122 full supertiles
TAIL = NR - NS * R               # 36 tail rows
N_VTZ = NS * 4 + 1               # VtZ matmul count (4 chunks/supertile + tail)
EPS = 1e-6

# permuted column order [U | T | V | Z] (original [U V Z T])
PERM = np.concatenate([np.arange(0, 32), np.arange(96, 128),
                       np.arange(32, 64), np.arange(64, 96)])

_CACHE = {}


def _build():
    nc = bacc.Bacc(None)
    x_in = nc.dram_tensor("xin", [NR, D_IN], F32, kind="ExternalInput")
    w_in = nc.dram_tensor("w2", [D_IN, 128], F32, kind="ExternalInput")
    b_in = nc.dram_tensor("b2", [128], F32, kind="ExternalInput")
    ccs_in = nc.dram_tensor("corrcs", [128], F32, kind="ExternalInput")
    cvtz_in = nc.dram_tensor("corrvtz", [KATT, KATT], F32, kind="ExternalInput")
    out_d = nc.dram_tensor("out", [NR, 2 * KATT], F32, kind="ExternalOutput")

    stats_in = nc.dram_tensor("stats_in", [1152], F32)
    stats_out = nc.dram_tensor("stats_out", [1152], F32, addr_space="Shared")

    with tile.TileContext(nc) as tc:
        with tc.tile_pool(name="const", bufs=1) as const, \
             tc.tile_pool(name="store", bufs=1) as store_p, \
             tc.tile_pool(name="small", bufs=1) as small, \
             tc.tile_pool(name="vtzps", bufs=1, space="PSUM") as vtzps:

            # ---- constants ----
            w_sb = const.tile([128, 2, 128], F32, tag="wsb")
            nc.sync.dma_start(w_sb[:], w_in.ap().rearrange("(g p) c -> p g c", g=2))
            w_r = const.tile([128, 2, 128], F32R, tag="wr")
            nc.vector.tensor_copy(w_r[:], w_sb[:])
            b_sb = const.tile([128, 1], F32, tag="bsb")
            nc.sync.dma_start(b_sb[:], b_in.ap().rearrange("(p o) -> p o", o=1))
            ident = const.tile([128, 128], F32, tag="ident")
            make_identity(nc, ident[:])
            identr = const.tile([128, 128], F32R, tag="identr")
            nc.vector.tensor_copy(identr[:], ident[:])
            ones1 = const.tile([1, 128], F32, tag="ones1")
            nc.gpsimd.memset(ones1[:], 1.0)
            csum = const.tile([128, 1], F32, tag="csum")
            nc.gpsimd.memset(csum[:], 0.0)
            ccs_sb = const.tile([128, 1], F32, tag="ccs")
            nc.sync.dma_start(ccs_sb[:], ccs_in.ap().rearrange("(p o) -> p o", o=1))
            cvtz_sb = const.tile([KATT, KATT], F32, tag="cvtz")
            nc.sync.dma_start(cvtz_sb[:], cvtz_in[:, :])

            # persistent UT store: pairs of supertiles share one [128,512] tile
            n_store = (NS + 1) // 2
            stores = [store_p.tile([128, R], F32R, tag=f"st{i}", name=f"st{i}")
                      for i in range(n_store)]
            store_tail = store_p.tile([128, 128], F32R, tag="sttail")

            vtz_ps = vtzps.tile([KATT, KATT], F32, tag="vtz")
            vtz_i = 0

            # ---------------- phase 1 ----------------
            with tc.tile_pool(name="p1sb", bufs=3) as p1sb, \
                 tc.tile_pool(name="p1sb2", bufs=2) as p1sb2, \
                 tc.tile_pool(name="p1ps", bufs=2, space="PSUM") as p1ps, \
                 tc.tile_pool(name="xtps", bufs=1, space="PSUM") as xtps:

                for s in range(NS):
                    x_sb = p1sb.tile([128, G * D_IN], F32, tag="xin")
                    nc.sync.dma_start(
                        x_sb[:],
                        x_in[s * R:(s + 1) * R, :].rearrange(
                            "(p g) d -> p (g d)", g=G))
                    xt_r = p1sb2.tile([128, 2, R], F32R, tag="xtr")
                    for k in range(2):
                        xt_ps = xtps.tile([128, R], F32, tag=f"xt{k}")
                        for g in range(G):
                            nc.tensor.transpose(
                                xt_ps[:, g * 128:(g + 1) * 128],
                                x_sb[:, g * D_IN + k * 128:g * D_IN + (k + 1) * 128],
                                ident[:])
                        nc.vector.tensor_copy(xt_r[:, k, :], xt_ps[:])

                    xt_mm = p1ps.tile([128, R], F32, tag="xmm")
                    nc.tensor.matmul(xt_mm[:], w_r[:, 0, :], xt_r[:, 0, :],
                                     start=True, stop=False)
                    nc.tensor.matmul(xt_mm[:], w_r[:, 1, :], xt_r[:, 1, :],
                                     start=False, stop=True)

                    xf = p1sb.tile([128, R], F32R, tag="xf")
                    cs_t = p1sb2.tile([128, 1], F32, tag="cst")
                    nc.scalar.activation(xf[:], xt_mm[:],
                                         mybir.ActivationFunctionType.Relu,
                                         bias=b_sb[:], accum_out=cs_t[:])
                    nc.vector.tensor_add(csum[:], csum[:], cs_t[:])

                    st = stores[s // 2]
                    off = (s % 2) * 64
                    nc.gpsimd.tensor_copy(st[off:off + 64, :], xf[0:64, :])

                    vz_ps = p1ps.tile([128, 4, 64], F32R, tag="vzps")
                    for c in range(G):
                        nc.tensor.transpose(
                            vz_ps[:, c, :], xf[64:128, c * 128:(c + 1) * 128],
                            identr[64:128, 64:128])
                    vz_sb = p1sb2.tile([128, 4, 64], F32R, tag="vzsb")
                    nc.scalar.copy(vz_sb[:], vz_ps[:])
                    for c in range(G):
                        nc.tensor.matmul(vtz_ps[:], vz_sb[:, c, 0:KATT],
                                         vz_sb[:, c, KATT:64],
                                         start=(vtz_i == 0),
                                         stop=(vtz_i == N_VTZ - 1))
                        vtz_i += 1

                # ---- tail tile (36 rows, padded with zeros) ----
                xt_sb = p1sb.tile([128, D_IN], F32, tag="xtail")
                nc.gpsimd.memset(xt_sb[:], 0.0)
                nc.sync.dma_start(xt_sb[0:TAIL, :], x_in[NS * R:NR, :])
                xtl_r = p1sb2.tile([128, 2, 128], F32R, tag="xtlr")
                for k in range(2):
                    xtl_ps = xtps.tile([128, 128], F32, tag=f"xt{k}")
                    nc.tensor.transpose(
                        xtl_ps[:], xt_sb[:, k * 128:(k + 1) * 128], ident[:])
                    nc.vector.tensor_copy(xtl_r[:, k, :], xtl_ps[:])
                xtl_mm = p1ps.tile([128, 128], F32, tag="xmm")
                nc.tensor.matmul(xtl_mm[:], w_r[:, 0, :], xtl_r[:, 0, :],
                                 start=True, stop=False)
                nc.tensor.matmul(xtl_mm[:], w_r[:, 1, :], xtl_r[:, 1, :],
                                 start=False, stop=True)
                xf_t = p1sb.tile([128, 128], F32R, tag="xftail")
                cs_t = p1sb2.tile([128, 1], F32, tag="cst")
                nc.scalar.activation(xf_t[:], xtl_mm[:],
                                     mybir.ActivationFunctionType.Relu,
                                     bias=b_sb[:], accum_out=cs_t[:])
                nc.vector.tensor_add(csum[:], csum[:], cs_t[:])
                nc.gpsimd.tensor_copy(store_tail[0:64, :], xf_t[0:64, :])
                vzt_ps = p1ps.tile([128, 64], F32R, tag="vzps")
                nc.tensor.transpose(vzt_ps[:], xf_t[64:128, :],
                                    identr[64:128, 64:128])
                vzt_sb = p1sb2.tile([128, 64], F32R, tag="vzsb")
                nc.scalar.copy(vzt_sb[:], vzt_ps[:])
                nc.tensor.matmul(vtz_ps[:], vzt_sb[:, 0:KATT],
                                 vzt_sb[:, KATT:64],
                                 start=False, stop=True)
                vtz_i += 1
                assert vtz_i == N_VTZ

            # ---------------- stats + collective ----------------
            vtz_sb = small.tile([KATT, KATT], F32, tag="vtzsb")
            nc.vector.tensor_copy(vtz_sb[:], vtz_ps[:])
            # subtract local pad pollution before the all-reduce
            nc.vector.tensor_sub(vtz_sb[:], vtz_sb[:], cvtz_sb[:])
            nc.vector.tensor_sub(csum[:], csum[:], ccs_sb[:])
            nc.sync.dma_start(
                stats_in[0:1024].rearrange("(p q) -> p q", p=KATT), vtz_sb[:])
            nc.sync.dma_start(
                stats_in[1024:1152].rearrange("(p q) -> p q", q=1), csum[:])
            nc.gpsimd.collective_compute(
                "AllReduce", mybir.AluOpType.add,
                replica_groups=[list(range(N_CORES))],
                ins=[stats_in.ap().opt()], outs=[stats_out.ap().opt()])
            u_sb = small.tile([KATT, 1], F32, tag="usb")
            s_sb = small.tile([KATT, 1], F32, tag="ssb")
            nc.sync.dma_start(
                u_sb[:], stats_out[1024:1056].rearrange("(p q) -> p q", q=1))
            nc.sync.dma_start(
                s_sb[:], stats_out[1088:1120].rearrange("(p q) -> p q", q=1))
            vtz_g = small.tile([128, KATT], F32, tag="vtzg")
            nc.sync.dma_start(
                vtz_g[0:KATT, :],
                stats_out[0:1024].rearrange("(p q) -> p q", p=KATT))
            nc.sync.dma_start(
                vtz_g[64:64 + KATT, :],
                stats_out[0:1024].rearrange("(p q) -> p q", p=KATT))

            with tc.tile_pool(name="dps", bufs=1, space="PSUM") as dps:
                us_ps = dps.tile([1, 1], F32, tag="us")
                nc.tensor.matmul(us_ps[:], u_sb[:], s_sb[:], start=True, stop=True)
                nf_sb = small.tile([1, 1], F32, tag="nf")
                nc.scalar.activation(nf_sb[:], us_ps[:],
                                     mybir.ActivationFunctionType.Copy,
                                     bias=EPS, scale=1.0 / N_TOTAL)
                d_sb = small.tile([1, 1], F32, tag="dsb")
                nc.vector.reciprocal(d_sb[:], nf_sb[:])
                d_ps = dps.tile([128, 1], F32, tag="dps")
                nc.tensor.matmul(d_ps[:], ones1[:], d_sb[:], start=True, stop=True)
                d_all = small.tile([128, 1], F32, tag="dall")
                nc.vector.tensor_copy(d_all[:], d_ps[:])
            vtz_d = small.tile([128, KATT], F32R, tag="vtzd")
            nc.vector.tensor_scalar_mul(vtz_d[:], vtz_g[:], d_all[:])

            # ---------------- phase 2 ----------------
            with tc.tile_pool(name="p2sb", bufs=3) as p2sb, \
                 tc.tile_pool(name="p2ps", bufs=2, space="PSUM") as p2ps:
                for s in range(NS):
                    st = stores[s // 2]
                    off = (s % 2) * 64
                    res_ps = p2ps.tile([KATT, R], F32, tag="res")
                    nc.tensor.matmul(res_ps[:], vtz_d[off:off + KATT, :],
                                     st[off:off + KATT, :],
                                     start=True, stop=True)
                    nc.scalar.copy(st[off:off + KATT, :], res_ps[:])
                    o_ps = p2ps.tile([128, 4, 64], F32R, tag="ops")
                    for c in range(G):
                        nc.tensor.transpose(
                            o_ps[:, c, :], st[off:off + 64, c * 128:(c + 1) * 128],
                            identr[off:off + 64, off:off + 64])
                    o_sb = p2sb.tile([128, 4, 64], F32, tag="osb")
                    nc.vector.tensor_copy(o_sb[:], o_ps[:])
                    nc.sync.dma_start(
                        out_d[s * R:(s + 1) * R, :].rearrange(
                            "(p g) q -> p g q", g=G), o_sb[:])
                # tail
                res_ps = p2ps.tile([KATT, 128], F32, tag="res")
                nc.tensor.matmul(res_ps[:], vtz_d[0:KATT, :],
                                 store_tail[0:KATT, :], start=True, stop=True)
                nc.scalar.copy(store_tail[0:KATT, :], res_ps[:])
                o_ps = p2ps.tile([128, 64], F32R, tag="ops")
                nc.tensor.transpose(o_ps[:], store_tail[0:64, :],
                                    identr[0:64, 0:64])
                o_sb = p2sb.tile([128, 64], F32, tag="osbt")
                nc.vector.tensor_copy(o_sb[:], o_ps[:])
                nc.sync.dma_start(out_d[NS * R:NR, :], o_sb[0:TAIL, :])

    nc.compile()
    return nc


def _prep_inputs(x, W, b):
    W = np.asarray(W, dtype=np.float32)
    b = np.asarray(b, dtype=np.float32)
    w2 = np.ascontiguousarray(W[:, PERM])
    b2 = np.ascontiguousarray(b[PERM])
    rb = np.maximum(b2, 0.0).astype(np.float32)
    n_pad = 128 - TAIL
    corrcs = (n_pad * rb).astype(np.float32)
    corrvtz = (n_pad * np.outer(rb[64:96], rb[96:128])).astype(np.float32)
    x = np.asarray(x, dtype=np.float32)
    in_maps = []
    for c in range(N_CORES):
        in_maps.append({
            "xin": np.ascontiguousarray(x[c * NR:(c + 1) * NR]),
            "w2": w2, "b2": b2, "corrcs": corrcs, "corrvtz": corrvtz,
        })
    return in_maps


def _run(x, W, b, trace=False):
    if "nc" not in _CACHE:
        _CACHE["nc"] = _build()
    nc = _CACHE["nc"]
    in_maps = _prep_inputs(x, W, b)
    res = run_bass_kernel_spmd(nc, in_maps, core_ids=list(range(N_CORES)),
                               trace=trace)
    out = np.concatenate([r["out"] for r in res.results], axis=0)
    return out, res


def kernel(x, W, b):
    out, _ = _run(x, W, b, trace=False)
    return out



# revision 2
# speedup vs baseline: 1.6507x; 1.6507x over previous
"""LowRankGlobalAttention TRN2 Bass kernel (8-core SPMD), v2.

out = concat(relu(xW+b)[:, :32] @ (V^T Z) * D, T) where
U,V,Z,T = relu(xW+b) column blocks, D = 1/(sum(U @ colsum(V))/N + eps).

v2 strategy (vs v1): x is converted to bf16 on the host, so the device
reads half the bytes AND can use the HBM->SBUF DMA-transpose (xbar) to
deliver x^T directly — no PE transposes of x, no PSUM round trips.
Output is written column-major as bf16 [64, NR] and transposed/upcast
on the host, so the device does no output transposes either.

Per core (62500 rows + pad to 62528 = 61 chunks of 1024 + one 64-row
tail chunk):
  Phase 1 per chunk: DMA-transpose x chunk -> xt [128, 2, 1024] bf16;
  W-stationary bf16 GEMM -> X^T psum [128, 1024]; relu on ACT (bias,
  accum_out colsums into per-chunk column); persist U^T/T^T halves in
  SBUF (bf16); PE-transpose the V^T/Z^T half back to row-major; 8
  accumulating VtZ matmuls into one psum tile.
  AllReduce tiny stats (VtZ [32,32] + colsums [128]); compute D.
  Phase 2 per chunk: res^T = (VtZ*D)-stationary matmul over stored U^T;
  overwrite the U^T slot with res^T; DMA [64, 1024] column-major out.
"""
import numpy as np
import ml_dtypes

import concourse.bass as bass
import concourse.mybir as mybir
import concourse.tile as tile
from concourse import bacc
from concourse.bass_utils import run_bass_kernel_spmd
from concourse.masks import make_identity

F32 = mybir.dt.float32
BF16 = mybir.dt.bfloat16
BF = ml_dtypes.bfloat16

N_CORES = 8
N_TOTAL = 500000
NR = N_TOTAL // N_CORES          # 62500 rows per core
D_IN = 256
KATT = 32
CH = 1024                        # chunk rows
NCH = NR // CH                   # 61 full chunks
TAILC = 64                       # padded tail chunk rows
NR_PAD = NCH * CH + TAILC        # 62528
EPS = 1e-6

# permuted column order [U | T | V | Z] (original [U V Z T])
PERM = np.concatenate([np.arange(0, 32), np.arange(96, 128),
                       np.arange(32, 64), np.arange(64, 96)])

_CACHE = {}


def _build(nr_pad=NR_PAD, nch=NCH, n_total=N_TOTAL, n_cores=N_CORES):
    nc = bacc.Bacc(None)
    x_in = nc.dram_tensor("xin", [nr_pad, D_IN], BF16, kind="ExternalInput")
    w_in = nc.dram_tensor("w2", [D_IN, 128], BF16, kind="ExternalInput")
    b_in = nc.dram_tensor("b2", [128], F32, kind="ExternalInput")
    ccs_in = nc.dram_tensor("corrcs", [128], F32, kind="ExternalInput")
    cvtz_in = nc.dram_tensor("corrvtz", [KATT, KATT], F32, kind="ExternalInput")
    out_d = nc.dram_tensor("out", [2 * KATT, nr_pad], BF16, kind="ExternalOutput")

    stats_in = nc.dram_tensor("stats_in", [1152], F32)
    stats_out = nc.dram_tensor("stats_out", [1152], F32, addr_space="Shared")

    n_vtz = nch * 8 + 1          # VtZ matmul count

    with tile.TileContext(nc) as tc:
        with tc.tile_pool(name="const", bufs=1) as const, \
             tc.tile_pool(name="store", bufs=1) as store_p, \
             tc.tile_pool(name="small", bufs=1) as small, \
             tc.tile_pool(name="vtzps", bufs=1, space="PSUM") as vtzps:

            # ---- constants ----
            w_sb = const.tile([128, 2, 128], BF16, tag="wsb")
            nc.sync.dma_start(w_sb[:], w_in.ap().rearrange("(k p) c -> p k c", k=2))
            b_sb = const.tile([128, 1], F32, tag="bsb")
            nc.sync.dma_start(b_sb[:], b_in.ap().rearrange("(p o) -> p o", o=1))
            ident_f = const.tile([128, 128], F32, tag="identf")
            make_identity(nc, ident_f[:])
            identb = const.tile([128, 128], BF16, tag="identb")
            nc.vector.tensor_copy(identb[:], ident_f[:])
            ones1 = const.tile([1, 128], F32, tag="ones1")
            nc.gpsimd.memset(ones1[:], 1.0)
            csbuf = const.tile([128, 64], F32, tag="csbuf")
            ccs_sb = const.tile([128, 1], F32, tag="ccs")
            nc.sync.dma_start(ccs_sb[:], ccs_in.ap().rearrange("(p o) -> p o", o=1))
            cvtz_sb = const.tile([KATT, KATT], F32, tag="cvtz")
            nc.sync.dma_start(cvtz_sb[:], cvtz_in[:, :])

            # persistent U^T/T^T store: chunk pairs share one [128,1024] tile
            n_store = (nch + 1) // 2
            stores = [store_p.tile([128, CH], BF16, tag=f"st{i}", name=f"st{i}")
                      for i in range(n_store)]
            store_tail = store_p.tile([128, TAILC], BF16, tag="sttail")

            vtz_ps = vtzps.tile([KATT, KATT], F32, tag="vtz")
            vtz_i = 0

            # ---------------- phase 1 ----------------
            with tc.tile_pool(name="p1xt", bufs=3) as p1xt, \
                 tc.tile_pool(name="p1xf", bufs=2) as p1xf, \
                 tc.tile_pool(name="p1vz", bufs=2) as p1vz, \
                 tc.tile_pool(name="p1mm", bufs=2, space="PSUM") as p1mm, \
                 tc.tile_pool(name="p1vzp", bufs=2, space="PSUM") as p1vzp:

                for i in range(nch + 1):
                    tail = i == nch
                    n = TAILC if tail else CH
                    xt = p1xt.tile([128, 2, CH], BF16, tag="xt")
                    nc.sync.dma_start(
                        xt[:, :, 0:n], x_in[i * CH:i * CH + n, :], transpose=True)

                    ps = p1mm.tile([128, CH], F32, tag="xmm")
                    nh = n // 2 if not tail else n
                    # W-stationary GEMM; keep same-stationary matmuls adjacent
                    for j in range(0, n, nh):
                        nc.tensor.matmul(ps[:, j:j + nh], w_sb[:, 0, :],
                                         xt[:, 0, j:j + nh],
                                         start=True, stop=False)
                    for j in range(0, n, nh):
                        nc.tensor.matmul(ps[:, j:j + nh], w_sb[:, 1, :],
                                         xt[:, 1, j:j + nh],
                                         start=False, stop=True)

                    xf = p1xf.tile([128, CH], BF16, tag="xf")
                    nc.scalar.activation(xf[:, 0:n], ps[:, 0:n],
                                         mybir.ActivationFunctionType.Relu,
                                         bias=b_sb[:],
                                         accum_out=csbuf[:, i:i + 1])

                    # persist U^T/T^T rows (partitions 0:64)
                    if tail:
                        nc.vector.tensor_copy(store_tail[0:64, :],
                                              xf[0:64, 0:n])
                    else:
                        st = stores[i // 2]
                        base = (i % 2) * 64
                        nc.vector.tensor_copy(st[base:base + 64, :],
                                              xf[0:64, :])

                    # V^T/Z^T back to row-major, then VtZ accumulation
                    vzp = p1vzp.tile([128, 8, 64], BF16, tag="vzp")
                    nb = n // 128 if not tail else 0
                    for c in range(nb):
                        nc.tensor.transpose(
                            vzp[:, c, :], xf[64:128, c * 128:(c + 1) * 128],
                            identb[64:128, 64:128])
                    if tail:
                        nc.tensor.transpose(vzp[0:n, 0, :], xf[64:128, 0:n],
                                            identb[64:128, 64:128])
                    vzs = p1vz.tile([128, 8, 64], BF16, tag="vzs")
                    if tail:
                        nc.vector.tensor_copy(vzs[0:n, 0, :], vzp[0:n, 0, :])
                        nc.tensor.matmul(vtz_ps[:], vzs[0:n, 0, 0:KATT],
                                         vzs[0:n, 0, KATT:64],
                                         start=(vtz_i == 0),
                                         stop=(vtz_i == n_vtz - 1))
                        vtz_i += 1
                    else:
                        nc.vector.tensor_copy(vzs[:], vzp[:])
                        for c in range(8):
                            nc.tensor.matmul(vtz_ps[:], vzs[:, c, 0:KATT],
                                             vzs[:, c, KATT:64],
                                             start=(vtz_i == 0),
                                             stop=(vtz_i == n_vtz - 1))
                            vtz_i += 1
                assert vtz_i == n_vtz

            # ---------------- stats + collective ----------------
            vtz_sb = small.tile([KATT, KATT], F32, tag="vtzsb")
            nc.vector.tensor_copy(vtz_sb[:], vtz_ps[:])
            nc.vector.tensor_sub(vtz_sb[:], vtz_sb[:], cvtz_sb[:])
            csum = small.tile([128, 1], F32, tag="csum")
            cs_dump = small.tile([128, 64], BF16, tag="csdump")
            nc.scalar.activation(cs_dump[:, 0:nch + 1], csbuf[:, 0:nch + 1],
                                 mybir.ActivationFunctionType.Copy,
                                 accum_out=csum[:])
            nc.vector.tensor_sub(csum[:], csum[:], ccs_sb[:])
            nc.sync.dma_start(
                stats_in[0:1024].rearrange("(p q) -> p q", p=KATT), vtz_sb[:])
            nc.sync.dma_start(
                stats_in[1024:1152].rearrange("(p q) -> p q", q=1), csum[:])
            nc.gpsimd.collective_compute(
                "AllReduce", mybir.AluOpType.add,
                replica_groups=[list(range(n_cores))],
                ins=[stats_in.ap().opt()], outs=[stats_out.ap().opt()])
            u_sb = small.tile([KATT, 1], F32, tag="usb")
            s_sb = small.tile([KATT, 1], F32, tag="ssb")
            nc.sync.dma_start(
                u_sb[:], stats_out[1024:1056].rearrange("(p q) -> p q", q=1))
            nc.sync.dma_start(
                s_sb[:], stats_out[1088:1120].rearrange("(p q) -> p q", q=1))
            vtz_g = small.tile([128, KATT], F32, tag="vtzg")
            nc.gpsimd.memset(vtz_g[:], 0.0)
            nc.sync.dma_start(
                vtz_g[0:KATT, :],
                stats_out[0:1024].rearrange("(p q) -> p q", p=KATT))
            nc.sync.dma_start(
                vtz_g[64:64 + KATT, :],
                stats_out[0:1024].rearrange("(p q) -> p q", p=KATT))

            with tc.tile_pool(name="dps", bufs=1, space="PSUM") as dps:
                us_ps = dps.tile([1, 1], F32, tag="us")
                nc.tensor.matmul(us_ps[:], u_sb[:], s_sb[:], start=True, stop=True)
                nf_sb = small.tile([1, 1], F32, tag="nf")
                nc.scalar.activation(nf_sb[:], us_ps[:],
                                     mybir.ActivationFunctionType.Copy,
                                     bias=EPS, scale=1.0 / n_total)
                d_sb = small.tile([1, 1], F32, tag="dsb")
                nc.vector.reciprocal(d_sb[:], nf_sb[:])
                d_ps = dps.tile([128, 1], F32, tag="dps")
                nc.tensor.matmul(d_ps[:], ones1[:], d_sb[:], start=True, stop=True)
                d_all = small.tile([128, 1], F32, tag="dall")
                nc.vector.tensor_copy(d_all[:], d_ps[:])
            vtzd = small.tile([128, KATT], BF16, tag="vtzd")
            nc.vector.tensor_scalar_mul(vtzd[:], vtz_g[:], d_all[:])

            # ---------------- phase 2 ----------------
            with tc.tile_pool(name="p2ps", bufs=2, space="PSUM") as p2ps:
                for i in range(nch):
                    st = stores[i // 2]
                    base = (i % 2) * 64
                    rps = p2ps.tile([KATT, CH], F32, tag="res")
                    for j in range(0, CH, 512):
                        nc.tensor.matmul(rps[:, j:j + 512],
                                         vtzd[base:base + KATT, :],
                                         st[base:base + KATT, j:j + 512],
                                         start=True, stop=True)
                    nc.vector.tensor_copy(st[base:base + KATT, :], rps[:])
                    nc.sync.dma_start(out_d[:, i * CH:(i + 1) * CH],
                                      st[base:base + 64, :])
                # tail
                rps = p2ps.tile([KATT, CH], F32, tag="res")
                nc.tensor.matmul(rps[:, 0:TAILC], vtzd[0:KATT, :],
                                 store_tail[0:KATT, :], start=True, stop=True)
                nc.vector.tensor_copy(store_tail[0:KATT, :], rps[:, 0:TAILC])
                nc.sync.dma_start(out_d[:, nch * CH:nch * CH + TAILC],
                                  store_tail[0:64, :])

    nc.compile()
    return nc


def _prep_inputs(x, W, b):
    W = np.asarray(W, dtype=np.float32)
    b = np.asarray(b, dtype=np.float32)
    w2 = np.ascontiguousarray(W[:, PERM]).astype(BF)
    b2 = np.ascontiguousarray(b[PERM]).astype(np.float32)
    rb = np.maximum(b2, 0.0).astype(np.float32)
    n_pad = NR_PAD - NR
    corrcs = (n_pad * rb).astype(np.float32)
    corrvtz = (n_pad * np.outer(rb[64:96], rb[96:128])).astype(np.float32)
    x = np.asarray(x, dtype=np.float32)
    in_maps = []
    for c in range(N_CORES):
        xc = np.zeros((NR_PAD, D_IN), dtype=BF)
        xc[:NR] = x[c * NR:(c + 1) * NR].astype(BF)
        in_maps.append({
            "xin": xc, "w2": w2, "b2": b2,
            "corrcs": corrcs, "corrvtz": corrvtz,
        })
    return in_maps


def _run(x, W, b, trace=False):
    if "nc" not in _CACHE:
        _CACHE["nc"] = _build()
    nc = _CACHE["nc"]
    in_maps = _prep_inputs(x, W, b)
    res = run_bass_kernel_spmd(nc, in_maps, core_ids=list(range(N_CORES)),
                               trace=trace)
    out = np.empty((N_TOTAL, 2 * KATT), dtype=np.float32)
    for c, r in enumerate(res.results):
        out[c * NR:(c + 1) * NR] = r["out"][:, :NR].T.astype(np.float32)
    return out, res


def kernel(x, W, b):
    out, _ = _run(x, W, b, trace=False)
    return out
